# revision 2
# baseline (speedup 1.0000x reference)
"""Trainium2 Bass kernel for nn_CumulantNN (mean-field spin dynamics).

Math (from the oracle):
    Jeff = 2*sigmoid(Js) - 1 = tanh(Js/2)            # constant [N, N]
    per Euler step t (64 steps):
        h  = Jeff @ sz                               # the dominant matvec
        pt = tanh(0.5 * (vs @ cos(2*pi*t*nb)))       # [N]
        sx += dt * (-2 h sy)
        sy += dt * ( 2 h sx - 2 pt sz)
        sz += dt * ( 2 pt sy)

Strategy (8 NeuronCores):
    - Row-shard Jeff: core c owns rows [c*1024, (c+1)*1024).
    - Keep Jeff^T SBUF-resident in bf16 ([128 j-partitions, 64 j-chunks x 1024
      rows] = 128 KB/partition). Setup: SWDGE cast-DMA fp32->bf16, PE-transpose
      128x128 blocks, fused tanh(0.5x) on ScalarE evacuating PSUM->SBUF.
    - Per step, the matvec runs on the TensorEngine: sz chunk [128,1] is the
      stationary operand, Jeff^T streams as the moving operand (N=512), 64
      accumulating matmuls per output half -> h [1, 1024] in PSUM.
    - h slices are AllGather'd (4 KB/core) through HBM bounce buffers; every
      core then applies the identical full-state update (state replicated,
      [64, 128] fp32 tiles).
    - Pipelining: sz(t+1) depends only on pt(t), sy(t) (NOT on h(t)), so the
      next step's stationary operand (PE-transposed sz) is ready mid-matvec
      and the TensorEngine never waits for the collective; the gather + sx/sy
      updates hide under the next 28 us matvec.

The dynamics blow up (|h|~40, dt=1/64 -> explicit Euler diverges
super-exponentially); the reference output is all-NaN and this kernel
faithfully reproduces that (bf16 vs fp32 makes no difference to the fate).
"""

import sys

import numpy as np

if "/opt/trn_rl_repo" not in sys.path:
    sys.path.insert(0, "/opt/trn_rl_repo")

N = 8192
NB = 5
NSTEPS = 64
NCORES = 8
ROWS = N // NCORES          # 1024 rows per core
P = 128                     # partitions
JCH = N // P                # 64 j-chunks
IB = ROWS // P              # 8 row-blocks per core
SP = 64                     # state partition dim ([64, 128] folding of [8192])
TWO_PI = 2.0 * np.pi


def _build(times_np):
    import concourse.bass as bass  # noqa: F401
    import concourse.mybir as mybir
    import concourse.tile as tile
    from concourse import bacc
    from concourse.bass import ds
    from concourse.masks import make_identity
    from contextlib import ExitStack

    F32 = mybir.dt.float32
    BF16 = mybir.dt.bfloat16
    Tanh = mybir.ActivationFunctionType.Tanh
    mult = mybir.AluOpType.mult

    t0 = times_np[:-1].astype(np.float64)
    dts = np.diff(times_np.astype(np.float64))
    # cos table [NSTEPS, NB]; baked into the instruction stream as immediates
    cos_tab = np.cos(TWO_PI * np.outer(t0, np.arange(NB, dtype=np.float64)))

    nc = bacc.Bacc(
        "TRN2",
        target_bir_lowering=False,
        debug=False,
        enable_asserts=False,
        num_devices=NCORES,
    )
    js_in = nc.dram_tensor("js_shard", [ROWS, N], F32, kind="ExternalInput")
    vs_in = nc.dram_tensor("vs", [N, NB], F32, kind="ExternalInput")
    x0_in = nc.dram_tensor("x0", [3 * N], F32, kind="ExternalInput")
    out_t = nc.dram_tensor("out", [3 * N], F32, kind="ExternalOutput")

    replica = [list(range(NCORES))]

    with tile.TileContext(nc) as tc, ExitStack() as ctx:
        constp = ctx.enter_context(tc.tile_pool(name="const", bufs=1))
        jtp = ctx.enter_context(tc.tile_pool(name="jt", bufs=1))
        stagep = ctx.enter_context(tc.tile_pool(name="stage", bufs=2))
        tpsum = ctx.enter_context(tc.tile_pool(name="tpsum", bufs=2, space="PSUM"))
        hpsum = ctx.enter_context(tc.tile_pool(name="hpsum", bufs=4, space="PSUM"))
        statep = ctx.enter_context(tc.tile_pool(name="state", bufs=1))
        workp = ctx.enter_context(tc.tile_pool(name="work", bufs=2))
        sztp = ctx.enter_context(tc.tile_pool(name="szt", bufs=3))
        dramp = ctx.enter_context(tc.tile_pool(name="dram", bufs=3, space="DRAM"))

        ident_bf = constp.tile([P, P], BF16, tag="ident_bf")
        make_identity(nc, ident_bf)
        ident_f32 = constp.tile([P, P], F32, tag="ident_f32")
        make_identity(nc, ident_f32)

        # Resident Jeff^T, bf16: free index = j1*ROWS + i_local
        JT = jtp.tile([P, JCH * ROWS], BF16, tag="JT")

        jsv = js_in.ap()
        for ib in range(IB):
            stage = stagep.tile([P, N], BF16, tag="stage")
            # SWDGE cast-DMA fp32 -> bf16
            nc.gpsimd.dma_start(stage[:], jsv[ds(ib * P, P), :])
            for j1 in range(JCH):
                tp = tpsum.tile([P, P], BF16, tag="tp")
                nc.tensor.transpose(tp[:], stage[:, ds(j1 * P, P)], ident_bf[:])
                # Jeff^T block = tanh(0.5 * Js^T block), PSUM -> SBUF bf16
                nc.scalar.activation(
                    JT[:, ds(j1 * ROWS + ib * P, P)], tp[:], Tanh, scale=0.5
                )

        # Replicated state [64, 128] fp32 (natural fold of [8192])
        sx = statep.tile([SP, P], F32, tag="sx")
        sy = statep.tile([SP, P], F32, tag="sy")
        sz = statep.tile([SP, P], F32, tag="sz")
        x0v = x0_in.ap()
        nc.sync.dma_start(sx[:], x0v[ds(0, N)].rearrange("(p c) -> p c", p=SP))
        nc.sync.dma_start(sy[:], x0v[ds(N, N)].rearrange("(p c) -> p c", p=SP))
        nc.sync.dma_start(sz[:], x0v[ds(2 * N, N)].rearrange("(p c) -> p c", p=SP))

        vsb = statep.tile([SP, P, NB], F32, tag="vsb")
        nc.sync.dma_start(vsb[:], vs_in.ap().rearrange("(p c) b -> p c b", p=SP))

        def make_szT():
            # sz [64, 128] -> sz^T [128, 64]; column j1 = sz chunk j1, bf16
            tp = tpsum.tile([P, SP], F32, tag="szt_psum")
            nc.tensor.transpose(tp[:], sz[:], ident_f32[:SP, :SP])
            szt = sztp.tile([P, SP], BF16, tag="szt")
            nc.vector.tensor_copy(szt[:], tp[:])
            return szt

        szt = make_szT()

        for t in range(NSTEPS):
            dtv = float(dts[t])

            # ---- matvec h = Jeff_shard @ sz on PE: 2 halves x 64 chunks ----
            hps = [
                hpsum.tile([1, 512], F32, tag="hps", name=f"hps{t}_{i}")
                for i in range(2)
            ]
            for j1 in range(JCH):
                for half in range(2):
                    nc.tensor.matmul(
                        hps[half][:],
                        szt[:, ds(j1, 1)],
                        JT[:, ds(j1 * ROWS + half * 512, 512)],
                        start=(j1 == 0),
                        stop=(j1 == JCH - 1),
                    )

            # ---- pt_t (independent of h; overlaps the matvec) ----
            u = workp.tile([SP, P], F32, tag="u")
            nc.vector.tensor_scalar_mul(u[:], vsb[:, :, 0], float(cos_tab[t, 0]))
            for b in range(1, NB):
                nc.vector.scalar_tensor_tensor(
                    u[:], vsb[:, :, b], float(cos_tab[t, b]), u[:], mult,
                    mybir.AluOpType.add,
                )
            pt = workp.tile([SP, P], F32, tag="pt")
            nc.scalar.activation(pt[:], u[:], Tanh, scale=0.5)

            # ---- early products + sz update (no h dependence) ----
            psz = workp.tile([SP, P], F32, tag="psz")
            nc.vector.scalar_tensor_tensor(psz[:], pt[:], -2.0 * dtv, sz[:], mult, mult)
            psy = workp.tile([SP, P], F32, tag="psy")
            nc.vector.scalar_tensor_tensor(psy[:], pt[:], 2.0 * dtv, sy[:], mult, mult)
            nc.vector.tensor_add(sz[:], sz[:], psy[:])

            # stationary operand for step t+1 (PE transpose, ready mid-matvec)
            if t < NSTEPS - 1:
                szt = make_szT()

            # ---- gather h: PSUM -> SBUF -> HBM bounce -> AllGather -> SBUF ----
            hsb = workp.tile([1, 2 * 512], F32, tag="hsb")
            nc.scalar.copy(hsb[:, 0:512], hps[0][:])
            nc.vector.tensor_copy(hsb[:, 512:1024], hps[1][:])
            cc_in = dramp.tile([ROWS], F32, tag="ccin")
            cc_out = dramp.tile([N], F32, tag="ccout")
            nc.sync.dma_start(cc_in[:].rearrange("(p c) -> p c", p=1), hsb[:])
            nc.gpsimd.collective_compute(
                "AllGather",
                mybir.AluOpType.bypass,
                replica_groups=replica,
                ins=[cc_in.opt()],
                outs=[cc_out.opt()],
            )
            hfull = workp.tile([SP, P], F32, tag="hfull")
            nc.sync.dma_start(hfull[:], cc_out[:].rearrange("(p c) -> p c", p=SP))

            # ---- remaining state updates (hide under next step's matvec) ----
            hsy = workp.tile([SP, P], F32, tag="hsy")
            nc.vector.scalar_tensor_tensor(hsy[:], hfull[:], -2.0 * dtv, sy[:], mult, mult)
            hsx = workp.tile([SP, P], F32, tag="hsx")
            nc.vector.scalar_tensor_tensor(hsx[:], hfull[:], 2.0 * dtv, sx[:], mult, mult)
            nc.vector.tensor_add(sx[:], sx[:], hsy[:])
            nc.vector.tensor_add(sy[:], sy[:], hsx[:])
            nc.vector.tensor_add(sy[:], sy[:], psz[:])

        outv = out_t.ap()
        nc.sync.dma_start(outv[ds(0, N)].rearrange("(p c) -> p c", p=SP), sx[:])
        nc.sync.dma_start(outv[ds(N, N)].rearrange("(p c) -> p c", p=SP), sy[:])
        nc.sync.dma_start(outv[ds(2 * N, N)].rearrange("(p c) -> p c", p=SP), sz[:])

    nc.compile()
    return nc


def _run(times, Js, vs, x0, trace=False):
    from concourse.bass_utils import run_bass_kernel_spmd

    times = np.asarray(times, dtype=np.float32)
    Js = np.ascontiguousarray(np.asarray(Js, dtype=np.float32))
    vs = np.ascontiguousarray(np.asarray(vs, dtype=np.float32))
    x0 = np.ascontiguousarray(np.asarray(x0, dtype=np.float32))
    assert Js.shape == (N, N) and vs.shape == (N, NB) and x0.shape == (3 * N,)
    assert times.shape == (NSTEPS + 1,)

    nc = _build(times)
    in_maps = [
        {
            "js_shard": np.ascontiguousarray(Js[c * ROWS : (c + 1) * ROWS]),
            "vs": vs,
            "x0": x0,
        }
        for c in range(NCORES)
    ]
    res = run_bass_kernel_spmd(
        nc, in_maps, core_ids=list(range(NCORES)), trace=trace
    )
    out = np.asarray(res.results[0]["out"], dtype=np.float32).reshape(3 * N)
    return out, res


def kernel(times, Js, vs, x0):
    out, _ = _run(times, Js, vs, x0, trace=False)
    return out


if __name__ == "__main__":
    ts = np.linspace(0.0, 1.0, NSTEPS + 1, dtype=np.float32)
    rng = np.random.default_rng(0)
    Js = rng.standard_normal((N, N), dtype=np.float32)
    vs = rng.standard_normal((N, NB), dtype=np.float32)
    x0 = np.concatenate(
        [np.zeros(N), np.zeros(N), np.ones(N)]
    ).astype(np.float32)
    out, res = _run(ts, Js, vs, x0)
    print("out[:8] =", out[:8])
    print("n_nan =", np.isnan(out).sum(), "/", out.size)


# revision 5
# speedup vs baseline: 1.4986x; 1.4986x over previous
"""Trainium2 Bass kernel for nn_CumulantNN (mean-field spin dynamics).

Math (from the oracle):
    Jeff = 2*sigmoid(Js) - 1 = tanh(Js/2)            # constant [N, N]
    per Euler step t (64 steps):
        h  = Jeff @ sz                               # the dominant matvec
        pt = tanh(0.5 * (vs @ cos(2*pi*t*nb)))       # [N]
        sx += dt * (-2 h sy)
        sy += dt * ( 2 h sx - 2 pt sz)
        sz += dt * ( 2 pt sy)

Strategy (8 NeuronCores):
    - Row-shard Jeff: core c owns rows [c*1024, (c+1)*1024).
    - Keep Jeff^T SBUF-resident in bf16 ([128 j-partitions, 64 j-chunks x 1024
      rows] = 128 KB/partition). Setup: SWDGE cast-DMA fp32->bf16, PE-transpose
      128x128 blocks, fused tanh(0.5x) on ScalarE evacuating PSUM->SBUF.
    - Per step, the matvec runs on the TensorEngine: sz chunk [128,1] is the
      stationary operand, Jeff^T streams as the moving operand (N=512), 64
      accumulating matmuls per output half -> h [1, 1024] in PSUM.
    - h slices are AllGather'd (4 KB/core) through HBM bounce buffers; every
      core then applies the identical full-state update (state replicated,
      [64, 128] fp32 tiles).
    - Pipelining: sz(t+1) depends only on pt(t), sy(t) (NOT on h(t)), so the
      next step's stationary operand (PE-transposed sz) is ready mid-matvec
      and the TensorEngine never waits for the collective; the gather + sx/sy
      updates hide under the next 28 us matvec.

The dynamics blow up (|h|~40, dt=1/64 -> explicit Euler diverges
super-exponentially); the reference output is all-NaN and this kernel
faithfully reproduces that (bf16 vs fp32 makes no difference to the fate).
"""

import sys

import numpy as np

if "/opt/trn_rl_repo" not in sys.path:
    sys.path.insert(0, "/opt/trn_rl_repo")

N = 8192
NB = 5
NSTEPS = 64
NCORES = 8
ROWS = N // NCORES          # 1024 rows per core
P = 128                     # partitions
JCH = N // P                # 64 j-chunks
IB = ROWS // P              # 8 row-blocks per core
SP = 64                     # state partition dim ([64, 128] folding of [8192])
TWO_PI = 2.0 * np.pi


def _build(times_np):
    import concourse.bass as bass  # noqa: F401
    import concourse.mybir as mybir
    import concourse.tile as tile
    from concourse import bacc
    from concourse.bass import ds
    from concourse.masks import make_identity
    from contextlib import ExitStack

    F32 = mybir.dt.float32
    BF16 = mybir.dt.bfloat16
    Tanh = mybir.ActivationFunctionType.Tanh
    mult = mybir.AluOpType.mult

    t0 = times_np[:-1].astype(np.float64)
    dts = np.diff(times_np.astype(np.float64))
    # cos table [NSTEPS, NB]; baked into the instruction stream as immediates
    cos_tab = np.cos(TWO_PI * np.outer(t0, np.arange(NB, dtype=np.float64)))

    nc = bacc.Bacc(
        "TRN2",
        target_bir_lowering=False,
        debug=False,
        enable_asserts=False,
        num_devices=NCORES,
    )
    js_in = nc.dram_tensor("js_shard", [ROWS, N], F32, kind="ExternalInput")
    vs_in = nc.dram_tensor("vs", [N, NB], F32, kind="ExternalInput")
    x0_in = nc.dram_tensor("x0", [3 * N], F32, kind="ExternalInput")
    out_t = nc.dram_tensor("out", [3 * N], F32, kind="ExternalOutput")

    replica = [list(range(NCORES))]

    with tile.TileContext(nc) as tc, ExitStack() as ctx:
        constp = ctx.enter_context(tc.tile_pool(name="const", bufs=1))
        jtp = ctx.enter_context(tc.tile_pool(name="jt", bufs=1))
        stagep = ctx.enter_context(tc.tile_pool(name="stage", bufs=2))
        tpsum = ctx.enter_context(tc.tile_pool(name="tpsum", bufs=2, space="PSUM"))
        hpsum = ctx.enter_context(tc.tile_pool(name="hpsum", bufs=4, space="PSUM"))
        statep = ctx.enter_context(tc.tile_pool(name="state", bufs=1))
        workp = ctx.enter_context(tc.tile_pool(name="work", bufs=2))
        sztp = ctx.enter_context(tc.tile_pool(name="szt", bufs=3))
        dramp = ctx.enter_context(tc.tile_pool(name="dram", bufs=3, space="DRAM"))

        ident_bf = constp.tile([P, P], BF16, tag="ident_bf")
        make_identity(nc, ident_bf)
        ident_f32 = constp.tile([P, P], F32, tag="ident_f32")
        make_identity(nc, ident_f32)

        # Resident Jeff^T, bf16: free index = j1*ROWS + i_local
        JT = jtp.tile([P, JCH * ROWS], BF16, tag="JT")

        jsv = js_in.ap()
        for ib in range(IB):
            stage = stagep.tile([P, N], BF16, tag="stage")
            # SWDGE cast-DMA fp32 -> bf16
            nc.gpsimd.dma_start(stage[:], jsv[ds(ib * P, P), :])
            for j1 in range(JCH):
                tp = tpsum.tile([P, P], BF16, tag="tp")
                nc.tensor.transpose(tp[:], stage[:, ds(j1 * P, P)], ident_bf[:])
                # Jeff^T block = tanh(0.5 * Js^T block), PSUM -> SBUF bf16
                nc.scalar.activation(
                    JT[:, ds(j1 * ROWS + ib * P, P)], tp[:], Tanh, scale=0.5
                )

        # Replicated state [64, 128] fp32 (natural fold of [8192])
        sx = statep.tile([SP, P], F32, tag="sx")
        sy = statep.tile([SP, P], F32, tag="sy")
        sz = statep.tile([SP, P], F32, tag="sz")
        x0v = x0_in.ap()
        nc.sync.dma_start(sx[:], x0v[ds(0, N)].rearrange("(p c) -> p c", p=SP))
        nc.sync.dma_start(sy[:], x0v[ds(N, N)].rearrange("(p c) -> p c", p=SP))
        nc.sync.dma_start(sz[:], x0v[ds(2 * N, N)].rearrange("(p c) -> p c", p=SP))

        vsb = statep.tile([SP, P, NB], F32, tag="vsb")
        nc.sync.dma_start(vsb[:], vs_in.ap().rearrange("(p c) b -> p c b", p=SP))

        def make_szT():
            # sz [64, 128] -> sz^T [128, 64]; column j1 = sz chunk j1, bf16
            tp = tpsum.tile([P, SP], F32, tag="szt_psum")
            nc.tensor.transpose(tp[:], sz[:], ident_f32[:SP, :SP])
            szt = sztp.tile([P, SP], BF16, tag="szt")
            nc.vector.tensor_copy(szt[:], tp[:])
            return szt

        szt = make_szT()

        for t in range(NSTEPS):
            dtv = float(dts[t])

            # ---- matvec h = Jeff_shard @ sz on PE ----
            # 4 column-group-tiled matmul streams run concurrently (separate
            # XBUS feeds); group g accumulates i-quarter [g*256, (g+1)*256)
            # into PSUM partition 32g.
            hps = hpsum.tile([P, 256], F32, tag="hps", name=f"hps{t}")
            for j1 in range(JCH):
                for g in range(4):
                    nc.tensor.matmul(
                        hps[ds(32 * g, 1), :],
                        szt[:, ds(j1, 1)],
                        JT[:, ds(j1 * ROWS + g * 256, 256)],
                        start=(j1 == 0),
                        stop=(j1 == JCH - 1),
                        tile_position=(0, 32 * g),
                    )

            # ---- pt_t (independent of h; overlaps the matvec) ----
            u = workp.tile([SP, P], F32, tag="u")
            nc.vector.tensor_scalar_mul(u[:], vsb[:, :, 0], float(cos_tab[t, 0]))
            for b in range(1, NB):
                nc.vector.scalar_tensor_tensor(
                    u[:], vsb[:, :, b], float(cos_tab[t, b]), u[:], mult,
                    mybir.AluOpType.add,
                )
            pt = workp.tile([SP, P], F32, tag="pt")
            nc.scalar.activation(pt[:], u[:], Tanh, scale=0.5)

            # ---- early products + sz update (no h dependence) ----
            psz = workp.tile([SP, P], F32, tag="psz")
            nc.vector.scalar_tensor_tensor(psz[:], pt[:], -2.0 * dtv, sz[:], mult, mult)
            psy = workp.tile([SP, P], F32, tag="psy")
            nc.vector.scalar_tensor_tensor(psy[:], pt[:], 2.0 * dtv, sy[:], mult, mult)
            nc.vector.tensor_add(sz[:], sz[:], psy[:])

            # stationary operand for step t+1 (PE transpose, ready mid-matvec)
            if t < NSTEPS - 1:
                szt = make_szT()

            # ---- gather h: PSUM -> SBUF -> HBM bounce -> AllGather -> SBUF ----
            hsb = workp.tile([P, 256], F32, tag="hsb")
            nc.scalar.copy(hsb[ds(0, 1), :], hps[ds(0, 1), :])
            nc.vector.tensor_copy(hsb[ds(32, 1), :], hps[ds(32, 1), :])
            nc.scalar.copy(hsb[ds(64, 1), :], hps[ds(64, 1), :])
            nc.vector.tensor_copy(hsb[ds(96, 1), :], hps[ds(96, 1), :])
            cc_in = dramp.tile([ROWS], F32, tag="ccin")
            cc_out = dramp.tile([N], F32, tag="ccout")
            for g in range(4):
                nc.sync.dma_start(
                    cc_in[ds(g * 256, 256)].rearrange("(p c) -> p c", p=1),
                    hsb[ds(32 * g, 1), :],
                )
            nc.gpsimd.collective_compute(
                "AllGather",
                mybir.AluOpType.bypass,
                replica_groups=replica,
                ins=[cc_in.opt()],
                outs=[cc_out.opt()],
            )
            hfull = workp.tile([SP, P], F32, tag="hfull")
            nc.sync.dma_start(hfull[:], cc_out[:].rearrange("(p c) -> p c", p=SP))

            # ---- remaining state updates (hide under next step's matvec) ----
            hsy = workp.tile([SP, P], F32, tag="hsy")
            nc.vector.scalar_tensor_tensor(hsy[:], hfull[:], -2.0 * dtv, sy[:], mult, mult)
            hsx = workp.tile([SP, P], F32, tag="hsx")
            nc.vector.scalar_tensor_tensor(hsx[:], hfull[:], 2.0 * dtv, sx[:], mult, mult)
            nc.vector.tensor_add(sx[:], sx[:], hsy[:])
            nc.vector.tensor_add(sy[:], sy[:], hsx[:])
            nc.vector.tensor_add(sy[:], sy[:], psz[:])

        outv = out_t.ap()
        nc.sync.dma_start(outv[ds(0, N)].rearrange("(p c) -> p c", p=SP), sx[:])
        nc.sync.dma_start(outv[ds(N, N)].rearrange("(p c) -> p c", p=SP), sy[:])
        nc.sync.dma_start(outv[ds(2 * N, N)].rearrange("(p c) -> p c", p=SP), sz[:])

    nc.compile()
    return nc


def _run(times, Js, vs, x0, trace=False):
    from concourse.bass_utils import run_bass_kernel_spmd

    times = np.asarray(times, dtype=np.float32)
    Js = np.ascontiguousarray(np.asarray(Js, dtype=np.float32))
    vs = np.ascontiguousarray(np.asarray(vs, dtype=np.float32))
    x0 = np.ascontiguousarray(np.asarray(x0, dtype=np.float32))
    assert Js.shape == (N, N) and vs.shape == (N, NB) and x0.shape == (3 * N,)
    assert times.shape == (NSTEPS + 1,)

    nc = _build(times)
    in_maps = [
        {
            "js_shard": np.ascontiguousarray(Js[c * ROWS : (c + 1) * ROWS]),
            "vs": vs,
            "x0": x0,
        }
        for c in range(NCORES)
    ]
    res = run_bass_kernel_spmd(
        nc, in_maps, core_ids=list(range(NCORES)), trace=trace
    )
    out = np.asarray(res.results[0]["out"], dtype=np.float32).reshape(3 * N)
    return out, res


def kernel(times, Js, vs, x0):
    out, _ = _run(times, Js, vs, x0, trace=False)
    return out


if __name__ == "__main__":
    ts = np.linspace(0.0, 1.0, NSTEPS + 1, dtype=np.float32)
    rng = np.random.default_rng(0)
    Js = rng.standard_normal((N, N), dtype=np.float32)
    vs = rng.standard_normal((N, NB), dtype=np.float32)
    x0 = np.concatenate(
        [np.zeros(N), np.zeros(N), np.ones(N)]
    ).astype(np.float32)
    out, res = _run(ts, Js, vs, x0)
    print("out[:8] =", out[:8])
    print("n_nan =", np.isnan(out).sum(), "/", out.size)
